# revision 37
# baseline (speedup 1.0000x reference)
"""MultiHeadClassifier (MoE routing) Trainium2 kernel.

Problem: B=65536 samples of dim D=1024, each routed by task_id to one of
T=16 two-layer heads (D->H=128 relu -> C=10). The dense reference computes
all 16 heads for every sample (275 GFLOP); here we route on the host and
compute only each sample's own head (~17 GFLOP), data-parallel with 2 tasks
per NeuronCore across 8 cores.

Strategy:
  - Host: stable-argsort samples by task; each core c owns tasks (2c, 2c+1).
    Every task segment is padded to a common M_task rows (multiple of 128) so
    the single SPMD program is identical across cores. Pad rows point at row 0
    (results discarded on unshard).
  - Host pre-transposes each core's gathered rows to xT [2, D, M_task] so the
    contraction dim D lies on SBUF partitions with contiguous DMA lines.
  - Device per (slot, m-tile of 512): 8 accumulating matmuls (W1 d-chunk
    [128,128] stationary, xT chunk [128,512] moving) -> PSUM [H=128, 512];
    ScalarE fused bias+ReLU into SBUF; one matmul with W2 [128,10] as lhsT ->
    PSUM [10,512]; ScalarE fused bias; DMA out [10, m] tiles.
  - Host scatters per-task outputs back to the original order.

MM_DTYPE selects the x/matmul precision (measured on trn2, 8 cores):
  - "fp8" (default): x cast to float8_e3m4 (TRN FP8_EXP3, 4 mantissa bits)
    scaled x2 with the inverse scale folded into the bf16 W1 stationary
    operand (mixed-dtype matmul). Halves the x DMA stream vs bf16 -> the
    kernel becomes PE-bound. The whole fp8 x for a core fits in SBUF, so
    all x DMAs are pre-issued into persistent tiles (no buffer-recycling
    stalls). Offline-measured rel err 1.38e-2 (threshold 2e-2).
  - "bf16": host casts x/W to bf16; DMA-bound. ~69 us, rel err ~3.4e-3.
  - "f32r": TF32-mode matmul. ~107 us, rel err ~2.1e-4.
  - "f32": exact fp32 two-pass matmul. ~150 us, rel err ~1.2e-7.
"""

import sys

import numpy as np

for _p in ("/opt/trn_rl_repo", "/root/.axon_site/_ro/trn_rl_repo"):
    if _p not in sys.path:
        sys.path.append(_p)

import concourse.bacc as bacc
import concourse.mybir as mybir
from concourse.bass_utils import run_bass_kernel_spmd
from concourse.tile import TileContext

B, D, T, H, C = 65536, 1024, 16, 128, 10
N_CORES = 8
S = T // N_CORES  # task slots per core = 2
DC = D // 128  # d-chunks of 128 = 8
MT = 512  # m-tile (max free dim into one PSUM bank)
X_BUFS = 2

MM_DTYPE = "fp8"

FP8_SCALE = 2.0  # x *= 2 on host; W1 /= 2 into the bf16 stationary operand
FP8_CLIP = 15.0  # e3m4 max normal is 15.5; clip below to avoid inf

# fp8 path: per-slot weight blob layout (bytes per partition row):
# w1 [DC*H bf16 = 2048] | b1 [f32 = 4] | w2 [C bf16 = 20] | b2 [f32 = 4] | pad
WB_W1, WB_B1, WB_W2, WB_B2, WB_PAD = 2048, 4, 20, 4, 4
WBYTES = WB_W1 + WB_B1 + WB_W2 + WB_B2 + WB_PAD  # 2080

_F32 = mybir.dt.float32
_BF16 = mybir.dt.bfloat16


def _mm_dt(mm_dtype):
    """(x dtype, w1 stationary dtype) on device."""
    return {
        "f32": (mybir.dt.float32, mybir.dt.float32),
        "f32r": (mybir.dt.float32r, mybir.dt.float32r),
        "bf16": (_BF16, _BF16),
        "fp8": (mybir.dt.float8e3, _BF16),
    }[mm_dtype]


def _np_in_dt(mm_dtype):
    import ml_dtypes

    return {
        "f32": np.dtype(np.float32),
        "f32r": np.dtype(np.float32),
        "bf16": np.dtype(ml_dtypes.bfloat16),
        "fp8": np.dtype(ml_dtypes.float8_e3m4),
    }[mm_dtype]


def _np_w_dt(mm_dtype):
    import ml_dtypes

    return (
        np.dtype(np.float32)
        if mm_dtype in ("f32", "f32r")
        else np.dtype(ml_dtypes.bfloat16)
    )


def _chunks(total, step, merge_tail=0):
    """Split [0, total) into (start, len) chunks of `step` plus remainder.

    A final chunk smaller than merge_tail is merged into the previous one.
    """
    out = []
    p = 0
    while p < total:
        c = min(step, total - p)
        out.append((p, c))
        p += c
    if merge_tail and len(out) > 1 and out[-1][1] < merge_tail:
        p0, c0 = out[-2]
        p1, c1 = out[-1]
        out[-2:] = [(p0, c0 + c1)]
    return out


def _fp8_blist(m_task):
    """fp8 block list, shared by _build and _prepare. 2048-col blocks with
    the tail merged into the last block (whose final 128-col subtile keeps
    the kernel-ending store tiny). A smaller 512-col head block was tried
    and measured neutral-to-worse (warmup already covers the first-chunk
    wait; single-sub waves have no ACT slack)."""
    return _chunks(m_task, 2048, merge_tail=512)


def _blocks(total, step):
    """Like _chunks but ends with a small (<=512) final block so the
    compute tail after the last DMA is short."""
    out = []
    p = 0
    rem = total
    while rem > 0:
        if rem <= 512 or rem <= step:
            c = rem
        elif rem <= step + 512:
            c = rem - 512
        else:
            c = step
        out.append((p, c))
        p += c
        rem -= c
    return out


def _build(M_task, mm_dtype=MM_DTYPE):
    dt_x, dt_w = _mm_dt(mm_dtype)
    # fp8: the whole per-core x fits in SBUF -> persistent tiles, every DMA
    # pre-issued, zero recycling deps. Other dtypes: rolling buffers.
    x_resident = mm_dtype == "fp8"
    XB = 2048
    x_bufs = 4 if mm_dtype == "bf16" else X_BUFS
    # bf16 matmuls double throughput when the HAM clock-gate is open; warmup
    # matmuls open it while the first x chunk streams in. The fp8 path only
    # needs to cover the ~3us first-chunk wait (warmup beyond that would
    # delay real matmuls, which keep the PE saturated from then on).
    n_warmup = 16 if mm_dtype == "bf16" else 10
    n_fill = 0
    nc = bacc.Bacc(None, target_bir_lowering=False)
    if x_resident:
        # dc-pair interleaved layout: for partition p, dc-pair q, the m-blocks
        # of dc=2q and dc=2q+1 alternate as contiguous runs, giving 4KB DMA
        # descriptor lines from 2KB (fp8) m-blocks. See _prepare.
        xT = nc.declare_dram_parameter(
            "xT", [S, DC // 2, 128, 2 * M_task], dt_x, isOutput=False
        )
        # all weights for a slot in ONE DMA (see WBYTES layout above): weight
        # loads go on the sync ring just ahead of the x stream, so their
        # completions can't cross-couple with x DMAs via shared sem lanes
        wb = nc.declare_dram_parameter("wb", [S, 128, WBYTES], mybir.dt.uint8,
                                       isOutput=False)
    else:
        xT = nc.declare_dram_parameter("xT", [S, D, M_task], dt_x, isOutput=False)
    dt_w2 = _BF16 if mm_dtype in ("bf16", "fp8") else dt_x
    if not x_resident:
        # w1 arrives host-repacked as [S, 128, DC*H]: partition-major
        w1 = nc.declare_dram_parameter("w1", [S, 128, DC * H], dt_w, isOutput=False)
        b1 = nc.declare_dram_parameter("b1", [S, H], _F32, isOutput=False)
        w2 = nc.declare_dram_parameter("w2", [S, H, C], dt_w2, isOutput=False)
        b2 = nc.declare_dram_parameter("b2", [S, C], _F32, isOutput=False)
    outT = nc.declare_dram_parameter("outT", [S, C, M_task], _F32, isOutput=True)

    relu = mybir.ActivationFunctionType.Relu
    h_dt = _BF16 if mm_dtype in ("bf16", "fp8") else dt_x

    use_warm = bool(n_warmup or n_fill)
    # PSUM banks: psum1 + psum2 = 8. Warmup matmuls borrow a psum2-ring slot
    # (they are guaranteed done before the second real mm2 needs it, since
    # the PE runs its queue in order).
    ps1_bufs = 5 if (use_warm and not x_resident) else 6

    from contextlib import ExitStack

    with TileContext(nc) as tc, ExitStack() as stack:
        wpool = stack.enter_context(tc.tile_pool(name="wpool", bufs=2))
        xpool = stack.enter_context(tc.tile_pool(name="xpool", bufs=x_bufs))
        hpool = stack.enter_context(tc.tile_pool(name="hpool", bufs=6))
        opool = stack.enter_context(tc.tile_pool(name="opool", bufs=2))
        psum1 = stack.enter_context(
            tc.tile_pool(name="psum1", bufs=ps1_bufs, space="PSUM")
        )
        psum2 = stack.enter_context(tc.tile_pool(name="psum2", bufs=2, space="PSUM"))
        if True:
            # PE warmup: dummy matmuls release the HAM clock-gate (~3.4us of
            # sustained PE busy) while the first x chunk streams in. The
            # memset goes on the vector engine: gpsimd is busy with SWDGE
            # descriptor-ring init at kernel start, which would delay the
            # warmup past the first x chunk and leave the PE cold for the
            # first ~10us of real matmuls.
            if use_warm:
                warm = stack.enter_context(tc.tile_pool(name="warm", bufs=1))
                wsrc = warm.tile([128, MT], _F32, tag="wsrc")
                (nc.vector if x_resident else nc.gpsimd).memset(wsrc[:], 0.0)
                wv = wsrc[:].bitcast(_BF16)
                if x_resident:
                    # ping-pong two psum2-ring tiles: a single warmup target
                    # serializes each matmul on the previous one's COMPLETION
                    # sem (~1.2us apart, leaving PE-idle gaps that delay the
                    # HAM clock-gate); alternating tiles makes them
                    # back-to-back (~0.43us cold)
                    wpss = [
                        psum2.tile([128, MT], _F32, tag="ps2", name=f"wps{i}")
                        for i in range(2)
                    ]
                else:
                    psumw = stack.enter_context(
                        tc.tile_pool(name="psumw", bufs=1, space="PSUM")
                    )
                    wpss = [psumw.tile([128, MT], _F32, tag="wps")]

            def fill_mm(n):
                for i in range(n):
                    wps = wpss[i % len(wpss)]
                    nc.tensor.matmul(
                        wps[:], wv[:, :128], wv[:, :MT], start=True, stop=True
                    )

            fill_mm(n_warmup)
            # interleave the slots' blocks (s0b0, s1b0, s0b1, ...); DMA issue
            # order == compute order so the PE chases one chunk behind
            if x_resident:
                blist = _fp8_blist(M_task)
            else:
                blist = _blocks(M_task, XB)
            works = [(blk, s) for blk in blist for s in range(S)]

            wts = []
            xtiles = {}
            n_store = [0]
            if x_resident:
                # one weight-blob DMA per slot on the sync ring, slot 0's
                # FIRST so the opening matmuls aren't gated on slot 1's
                wbt = {}
                for s in range(S):
                    t = wpool.tile(
                        [128, WBYTES], mybir.dt.uint8, tag=f"wb{s}", bufs=1,
                        name=f"wbt{s}",
                    )
                    wbt[s] = t
                    o1 = WB_W1
                    o2 = o1 + WB_B1
                    o3 = o2 + WB_W2
                    wts.append((
                        t[:, :o1].bitcast(dt_w).rearrange(
                            "p (dc h) -> p dc h", dc=DC
                        ),
                        t[:, o1:o2].bitcast(_F32),
                        t[:, o2:o3].bitcast(dt_w2),
                        # b2 replicated on 4 partition bands (32j..32j+C) for
                        # the column-tiled mm2 whose outputs land per-band
                        [
                            t[32 * j : 32 * j + C, o3 : o3 + WB_B2].bitcast(_F32)
                            for j in range(4)
                        ],
                    ))
                nc.sync.dma_start(wbt[0], wb[0])
                # pre-issue EVERY x-chunk DMA into its own persistent tile:
                # the sync ring streams them back-to-back for the whole run.
                # A chunk is a dc-PAIR [128, 2*xl] (4KB descriptor lines).
                for wi, ((x0, xl), s) in enumerate(works):
                    if wi == 1:
                        nc.sync.dma_start(wbt[1], wb[1])
                    for q in range(DC // 2):
                        xtc = xpool.tile(
                            [128, 2 * xl],
                            dt_x,
                            tag=f"x{q}w{wi}",
                            bufs=1,
                            name=f"x_{q}_{wi}",
                        )
                        if wi == 0 and q == 0:
                            # split the very first chunk at the dc0/dc1 run
                            # boundary: the first matmul waits only dc0's
                            # completion sem (DMA receipt is ~2us, so a
                            # smaller first transfer starts the PE ~1.4us
                            # earlier)
                            nc.sync.dma_start(
                                xtc[:, :xl], xT[s, q, :, 2 * x0 : 2 * x0 + xl]
                            )
                            nc.sync.dma_start(
                                xtc[:, xl:],
                                xT[s, q, :, 2 * x0 + xl : 2 * (x0 + xl)],
                            )
                        else:
                            nc.sync.dma_start(
                                xtc, xT[s, q, :, 2 * x0 : 2 * (x0 + xl)]
                            )
                        xtiles[(wi, q)] = xtc
            else:
                # hoist BOTH slots' weight loads to kernel start, on the
                # scalar HWDGE ring: they neither queue behind the x-chunk
                # stream (sync ring) nor behind slot-0's relu work
                for s in range(S):
                    w1t = wpool.tile([128, DC, H], dt_w, tag="w1", name=f"w1t{s}")
                    nc.scalar.dma_start(
                        w1t, w1[s].rearrange("p (dc h) -> p dc h", dc=DC)
                    )
                    b1t = wpool.tile([H, 1], _F32, tag="b1", name=f"b1t{s}")
                    nc.scalar.dma_start(b1t, b1[s][:, None])
                    w2t = wpool.tile([H, C], dt_w2, tag="w2", name=f"w2t{s}")
                    nc.scalar.dma_start(w2t, w2[s])
                    b2t = wpool.tile([C, 1], _F32, tag="b2", name=f"b2t{s}")
                    nc.scalar.dma_start(b2t, b2[s][:, None])
                    wts.append((w1t, b1t, w2t, b2t))

            for wi, ((x0, xl), s) in enumerate(works):
                w1t, b1t, w2t, b2t = wts[s]
                if x_resident:
                    # dc -> (pair tile, column offset of the dc's run)
                    xts = [
                        (xtiles[(wi, dc // 2)], (dc % 2) * xl) for dc in range(DC)
                    ]
                else:
                    xT_s = xT[s].rearrange("(dc p) m -> p dc m", p=128)
                    # per-d-chunk tiles/DMAs: contiguous descriptors AND
                    # chunk-granular deps, so matmuls start on partial data
                    xts = []
                    for dc in range(DC):
                        xtc = xpool.tile(
                            [128, XB + 384], dt_x, tag=f"x{dc}", name=f"x_{dc}"
                        )
                        nc.sync.dma_start(xtc[:, :xl], xT_s[:, dc, x0 : x0 + xl])
                        xts.append((xtc, 0))
                subs = _chunks(xl, MT)
                last_work = wi == len(works) - 1
                if not x_resident:
                    ot = opool.tile([C, XB + 384], _F32, tag="o")
                # waves of <=4 m-subtiles (PSUM bank budget); within a
                # wave loop dc-outer so subtiles run back-to-back on the
                # same stationary W1 chunk.
                for w0 in range(0, len(subs), 4):
                    wave = subs[w0 : w0 + 4]
                    ps1s = [
                        psum1.tile([H, MT], _F32, tag="ps1", name=f"ps1_{j}")
                        for j in range(len(wave))
                    ]
                    last_wave = last_work and w0 + 4 >= len(subs)
                    for dc in range(DC):
                        xtc, xoff = xts[dc]
                        for j, (m0, mt) in enumerate(wave):
                            nc.tensor.matmul(
                                ps1s[j][:, :mt],
                                w1t[:, dc, :],
                                xtc[:, xoff + m0 : xoff + m0 + mt],
                                start=(dc == 0),
                                stop=(dc == DC - 1),
                            )
                        if not (last_wave and dc == DC - 1):
                            fill_mm(n_fill)
                    if x_resident:
                        otw = opool.tile(
                            [C, MT * 4], _F32, tag=f"ow{wi}_{w0 // 4}", bufs=1,
                            name=f"ot_{wi}_{w0}",
                        )
                        for j, (m0, mt) in enumerate(wave):
                            ht = hpool.tile([H, MT], h_dt, tag="h")
                            nc.scalar.activation(
                                ht[:, :mt], ps1s[j][:, :mt], relu, bias=b1t
                            )
                            ps2 = psum2.tile([C, MT], _F32, tag="ps2")
                            nc.tensor.matmul(
                                ps2[:, :mt], w2t, ht[:, :mt], start=True,
                                stop=True,
                            )
                            if last_wave:
                                # ScalarE Identity+bias instead of DVE add:
                                # skips the DVE queue behind the previous
                                # wave's ~0.67us tensor_tensors, so the
                                # kernel-ending store issues ~1us earlier
                                nc.scalar.activation(
                                    otw[:, j * MT : j * MT + mt],
                                    ps2[:, :mt],
                                    mybir.ActivationFunctionType.Identity,
                                    bias=b2t[0],
                                )
                            else:
                                nc.vector.tensor_tensor(
                                    otw[:, j * MT : j * MT + mt],
                                    ps2[:, :mt],
                                    b2t[0].to_broadcast([C, mt]),
                                    mybir.AluOpType.add,
                                )
                        # per-wave stores on the scalar HWDGE ring: using
                        # gpsimd (SWDGE) forces a ~2.6us Q7 descriptor-ring
                        # drain before gpsimd can join the final barrier, and
                        # SWDGE completion is ~1us slower per store. Scalar
                        # reaches each store right after that wave's ACTs, so
                        # the tt-wait is short and ACT slack (~3.5us/wave)
                        # absorbs it. The kernel-ending store (128 cols) uses
                        # the by-then-idle sync ring.
                        wlen = sum(mt for _, mt in wave)
                        dst = outT[s, :, x0 + w0 * MT : x0 + w0 * MT + wlen]
                        eng = nc.sync if last_wave else nc.scalar
                        eng.dma_start(dst, otw[:, :wlen])
                    else:
                        for j, (m0, mt) in enumerate(wave):
                            ht = hpool.tile([H, MT], h_dt, tag="h")
                            nc.scalar.activation(
                                ht[:, :mt], ps1s[j][:, :mt], relu, bias=b1t
                            )
                            ps2 = psum2.tile([C, MT], _F32, tag="ps2")
                            nc.tensor.matmul(
                                ps2[:, :mt], w2t, ht[:, :mt], start=True,
                                stop=True,
                            )
                            nc.vector.tensor_tensor(
                                ot[:, m0 : m0 + mt],
                                ps2[:, :mt],
                                b2t.to_broadcast([C, mt]),
                                mybir.AluOpType.add,
                            )
                if not x_resident:
                    # gpsimd (SWDGE): keeps the waiting out-DMA off the SP
                    # HWDGE ring so it can't head-of-line block x-chunk DMAs
                    nc.gpsimd.dma_start(outT[s, :, x0 : x0 + xl], ot[:, :xl])
    nc.compile()
    return nc


def _prepare(x, task_id, W1, b1, W2, b2, mm_dtype=MM_DTYPE):
    """Host-side routing: returns (in_maps, idx, counts, M_task)."""
    np_x = _np_in_dt(mm_dtype)
    np_w = _np_w_dt(mm_dtype)
    x = np.ascontiguousarray(np.asarray(x, dtype=np.float32))
    task_id = np.asarray(task_id).astype(np.int64)
    W1 = np.asarray(W1, dtype=np.float32)
    b1 = np.asarray(b1, dtype=np.float32)
    W2 = np.asarray(W2, dtype=np.float32)
    b2 = np.asarray(b2, dtype=np.float32)

    if mm_dtype == "fp8":
        W1 = W1 / FP8_SCALE  # exact inverse of the host x scale (power of 2)

    order = np.argsort(task_id, kind="stable")
    counts = np.bincount(task_id, minlength=T)
    starts = np.concatenate([[0], np.cumsum(counts)])
    M_task = max(128, int(-(-int(counts.max()) // 128) * 128))

    # idx[t] = sample rows for task t, padded with row 0 (discarded later)
    idx = np.zeros((T, M_task), dtype=np.int64)
    for t in range(T):
        idx[t, : counts[t]] = order[starts[t] : starts[t + 1]]

    if mm_dtype == "fp8":
        blist = _fp8_blist(M_task)

    in_maps = []
    for c in range(N_CORES):
        ts_c = [S * c + s for s in range(S)]
        rows = idx[ts_c].reshape(-1)  # [S * M_task]
        xg = x[rows].reshape(S, M_task, D)
        xT = np.ascontiguousarray(xg.transpose(0, 2, 1))
        if mm_dtype == "fp8":
            xT = np.clip(xT * FP8_SCALE, -FP8_CLIP, FP8_CLIP).astype(np_x)
            # dc-pair interleave (must mirror _build's XB/blist): for each
            # m-block, partition p, dc-pair q: runs of dc=2q then dc=2q+1
            # back-to-back -> 4KB contiguous DMA descriptor lines.
            x4 = xT.reshape(S, DC // 2, 2, 128, M_task)
            parts = [
                x4[:, :, :, :, x0 : x0 + xl]
                .transpose(0, 1, 3, 2, 4)  # [S, 4, 128, 2, xl]
                .reshape(S, DC // 2, 128, 2 * xl)
                for (x0, xl) in blist
            ]
            xT = np.ascontiguousarray(np.concatenate(parts, axis=3))
        else:
            xT = xT.astype(np_x)
        # repack W1 [D, H] -> [128, DC*H] (partition-major for 4KB DMA rows)
        w1p = (
            W1[ts_c]
            .reshape(S, DC, 128, H)
            .transpose(0, 2, 1, 3)
            .reshape(S, 128, DC * H)
        ).astype(np_w)
        b1p = np.ascontiguousarray(b1[ts_c]).astype(np.float32)
        w2p = np.ascontiguousarray(W2[ts_c]).astype(
            _np_w_dt("bf16") if mm_dtype in ("bf16", "fp8") else np_w
        )
        b2p = np.ascontiguousarray(b2[ts_c]).astype(np.float32)
        if mm_dtype == "fp8":
            # per-slot weight blob [S, 128, WBYTES] u8 (see kernel layout)
            blob = np.zeros((S, 128, WBYTES), dtype=np.uint8)
            o1, o2, o3 = WB_W1, WB_W1 + WB_B1, WB_W1 + WB_B1 + WB_W2
            blob[:, :, :o1] = w1p.view(np.uint8)
            blob[:, :, o1:o2] = b1p[:, :, None].view(np.uint8)
            blob[:, :, o2:o3] = w2p.view(np.uint8).reshape(S, 128, WB_W2)
            # b2 on 4 partition bands (32j..32j+C) for the col-tiled mm2
            for j in range(4):
                blob[:, 32 * j : 32 * j + C, o3 : o3 + WB_B2] = b2p[
                    :, :, None
                ].view(np.uint8)
            in_maps.append({"xT": xT, "wb": blob})
        else:
            in_maps.append(
                {
                    "xT": xT,
                    "w1": np.ascontiguousarray(w1p),
                    "b1": b1p,
                    "w2": w2p,
                    "b2": b2p,
                }
            )
    return in_maps, idx, counts, M_task


def _unshard(results, idx, counts, b_total=B):
    out = np.empty((b_total, C), dtype=np.float32)
    for c in range(N_CORES):
        yT = np.asarray(results[c]["outT"])  # [S, C, M_task]
        y = yT.transpose(0, 2, 1)  # [S, M_task, C]
        for s in range(S):
            t = S * c + s
            cnt = counts[t]
            out[idx[t, :cnt]] = y[s, :cnt]
    return out


def kernel(x, task_id, W1, b1, W2, b2):
    in_maps, idx, counts, M_task = _prepare(x, task_id, W1, b1, W2, b2)
    nc = _build(M_task)
    try:
        res = run_bass_kernel_spmd(nc, in_maps, list(range(N_CORES)))
    except Exception:
        # transient NRT device hiccups (e.g. NRT_EXEC_UNIT_UNRECOVERABLE)
        # have been observed to succeed on retry
        res = run_bass_kernel_spmd(nc, in_maps, list(range(N_CORES)))
    return _unshard(res.results, idx, counts, b_total=np.asarray(task_id).shape[0])
